# revision 6
# baseline (speedup 1.0000x reference)
"""CRF negative log-likelihood on 8 Trainium2 NeuronCores — v2.

Spliced forward algorithm, exp domain:  P_{t+1} = q_{t+1} * (M P_t)  with
q = exp(feat - DELTA), M = exp(transitions).  The positive-matrix product
contracts fast (Hilbert metric), so the 511 transitions split into ~64
segments whose initial directions the HOST computes (Perron vector + 2
warmup steps + 1 folded transition, f64); per-batch logZ is stitched on
the host from each segment's final state (rank-1 splicing, validated
err ~3e-3 against exact; tolerance ~49 absolute).

Device work per chain slot per unit (states bf16, 96 partitions = 2
segments stacked; free cols = blocks x 128 batch):
  PE   matmul [96x96 blockdiag exp(T)] @ [96,<=512] -> PSUM f32
  then one of two evacuation paths (DVE/Act capacity split):
   D:  DVE tensor_tensor (PSUM x q) -> SBUF bf16            (direct)
   A:  Act copy PSUM -> SBUF bf16; DVE bf16 fast mult by q  (copy+defer)

1 D-unit [96,1024] (L=9) + 3 A-units [96,1024] (L=6), PSUM exactly
8 banks.  q is precomputed on host (exp + layout) and streamed bf16 in
2-slot chunks; per-unit fp8 [init | q-slot0] lead DMAs (max-normalized
inits, DELTA0=0 q-slot0, exact scale bookkeeping host-side) halve the
serialized DMA prologue so every chain starts early; each unit's final
slot ships the raw (pre q-mult) state and the host applies the last q
during stitching, saving the final evacuation mults.
Gold path score and the final mean are host-side, as in the baseline.
"""

import numpy as np
import ml_dtypes

B, S, T = 1024, 512, 48
NCORES = 8
BC = B // NCORES          # 128 batch rows per core
DELTA = 5.0
P2 = 2 * T                # 96 partitions = 2 segments stacked

# units: (kind, width, L_device_slots); 'D' = DVE direct mult,
# 'A' = Act copy + deferred DVE mult.  A unit of width w holds w/64
# segments (2 stacked x w/128 blocks); each segment covers 1 host-folded
# transition + L device transitions.
UNIT_SPEC = [("D", 1024, 9),
             ("A", 1024, 6), ("A", 1024, 6), ("A", 1024, 6)]
NUNIT = len(UNIT_SPEC)
DEV_TRANS = sum((w // 64) * L for _, w, L in UNIT_SPEC)
COVER = sum((w // 64) * (L + 1) for _, w, L in UNIT_SPEC)
JPRE = (S - 1) - COVER                       # host prefix transitions
QCOLS = sum(w * (L - 1) for _, w, L in UNIT_SPEC)
UOFF = [0]
for _k, _w, _L in UNIT_SPEC:
    UOFF.append(UOFF[-1] + _w)
OUTCOLS = UOFF[-1]
LEADW = sum(2 * w for _, w, _L in UNIT_SPEC)   # fp8 [init|q0] per unit
NSEG = sum(w // 64 for _, w, _L in UNIT_SPEC)
WHOST = 2                 # host warmup steps per segment init
assert JPRE >= 0, (JPRE, COVER)

BF16 = ml_dtypes.bfloat16

_NC = None
_SEG = None


def _segments():
    """seg_id -> (unit, block j, half h, fold_t, L). seg order: unit-major,
    then block, then half.  Transition fold_t is applied on the host (its
    growth counted there); device applies fold_t+1 .. fold_t+L."""
    global _SEG
    if _SEG is None:
        segs = []
        t0 = JPRE + 1
        for u, (_, w, L) in enumerate(UNIT_SPEC):
            for j in range(w // BC):
                for h in range(2):
                    segs.append((u, j, h, t0, L))
                    t0 += L + 1
        assert t0 == S, (t0, S)
        _SEG = segs
    return _SEG


def _build_nc():
    import concourse.mybir as mybir
    import concourse.tile as tile
    from concourse import bacc

    f32 = mybir.dt.float32
    bf16 = mybir.dt.bfloat16
    Act = mybir.ActivationFunctionType
    Alu = mybir.AluOpType

    nc = bacc.Bacc()

    fp8 = mybir.dt.float8e4
    # q carries e2 in its first P2 cols; leads (init|q0 per unit) are fp8
    q_d = nc.declare_dram_parameter("q", [P2, P2 + QCOLS], bf16,
                                    isOutput=False)
    lead_d = nc.declare_dram_parameter("lead", [P2, LEADW], fp8,
                                       isOutput=False)
    # final outputs are raw (pre q-mult) states: Act copies PSUM -> SBUF
    # bf16, DMA ships them, host applies the last q during stitching --
    # saves every unit's final DVE mult
    outs_d = nc.declare_dram_parameter("outs", [P2, OUTCOLS], bf16,
                                       isOutput=True)

    Ks = [k for k, _, _ in UNIT_SPEC]
    Ws = [w for _, w, _ in UNIT_SPEC]
    Ls = [L for _, _, L in UNIT_SPEC]
    # device consumes q only for slots 0..L-2 (last slot ships raw PSUM)
    qoff = np.cumsum([0] + [Ws[u] * (Ls[u] - 1)
                            for u in range(NUNIT)]).tolist()
    loff = np.cumsum([P2] + [2 * w for w in Ws]).tolist()
    CHUNK = 4  # slots per q DMA

    with tile.TileContext(nc) as tc:
        with (
            tc.tile_pool(name="const", bufs=1) as cpool,
            tc.tile_pool(name="qp", bufs=1) as qpool,
            tc.tile_pool(name="st", bufs=2) as spool,
            tc.tile_pool(name="raw", bufs=2) as rpool,
            tc.tile_pool(name="psum", bufs=1, space="PSUM") as psum,
        ):
            # fp8 lead tile [(init_u, q0_u) x units]; one DMA per unit
            lead_sb = cpool.tile([P2, LEADW], fp8, name="lead_sb")
            e2_sb = cpool.tile([P2, P2], bf16, name="e2_sb")

            # static q tiles, one per unit, chunk-DMA'd into subranges
            q_sb = [qpool.tile([P2, Ls[u] * Ws[u]], bf16, tag=f"q{u}",
                               name=f"q{u}") for u in range(NUNIT)]

            def load_q(u, lo, hi):
                hi = min(hi, Ls[u] - 1)
                if lo >= hi:
                    return
                w = Ws[u]
                nc.sync.dma_start(
                    q_sb[u][:, lo * w:hi * w],
                    q_d[:, qoff[u] + lo * w:qoff[u] + hi * w])

            nc.sync.dma_start(e2_sb[:, :], q_d[:, 0:P2])
            for u in range(NUNIT):
                nc.sync.dma_start(lead_sb[:, loff[u]:loff[u + 1]],
                                  lead_d[:, loff[u]:loff[u + 1]])
            for u in range(NUNIT):
                load_q(u, 1, 2)
            for u in range(NUNIT):
                load_q(u, 2, 3)
            nchunk = max((L + CHUNK - 1) // CHUNK for L in Ls)
            for c in range(nchunk):
                for u in range(NUNIT):
                    load_q(u, 3 + c * CHUNK, 3 + (c + 1) * CHUNK)

            pt = [psum.tile([P2, Ws[u]], f32, name=f"p{u}")
                  for u in range(NUNIT)]

            states = [lead_sb[:, loff[u]:loff[u] + Ws[u]]
                      for u in range(NUNIT)]

            maxL = max(Ls)
            for s in range(maxL):
                for u in range(NUNIT):
                    kind, w, L = UNIT_SPEC[u]
                    if s >= L:
                        continue
                    prev = states[u]
                    for r in range(0, w, 512):
                        r2 = min(r + 512, w)
                        nc.tensor.matmul(pt[u][:, r:r2], e2_sb[:, :],
                                         prev[:, r:r2], start=True, stop=True)
                    if s == L - 1:
                        # last slot: Act-copy raw out; host applies final q
                        rw = rpool.tile([P2, w], bf16, tag=f"rw{u}",
                                        name=f"rw{u}_fin")
                        nc.scalar.activation(rw[:, :], pt[u][:, :], Act.Copy)
                        nc.sync.dma_start(outs_d[:, UOFF[u]:UOFF[u + 1]],
                                          rw[:, :])
                        continue
                    if s == 0:
                        qs = lead_sb[:, loff[u] + w:loff[u] + 2 * w]
                    else:
                        qs = q_sb[u][:, s * w:(s + 1) * w]
                    stt = spool.tile([P2, w], bf16, tag=f"st{u}",
                                     name=f"st{u}_{s}")
                    if kind == "D":
                        nc.vector.tensor_tensor(stt[:, :], pt[u][:, :], qs,
                                                Alu.mult)
                    else:
                        rw = rpool.tile([P2, w], bf16, tag=f"rw{u}",
                                        name=f"rw{u}_{s}")
                        nc.scalar.activation(rw[:, :], pt[u][:, :], Act.Copy)
                        nc.vector.tensor_tensor(stt[:, :], rw[:, :], qs,
                                                Alu.mult)
                    states[u] = stt

    if not nc.is_finalized():
        nc.finalize()
    return nc


def _get_nc():
    global _NC
    if _NC is None:
        _NC = _build_nc()
    return _NC


def _host_gold(feats, tags, Tm, st, sp):
    emit = np.take_along_axis(feats, tags[..., None], axis=2)[..., 0]
    gold = (
        emit.sum(axis=1, dtype=np.float64)
        + Tm[tags[:, 1:], tags[:, :-1]].sum(axis=1, dtype=np.float64)
        + st[tags[:, 0]].astype(np.float64)
        + sp[tags[:, -1]].astype(np.float64)
    )
    return gold


def kernel(feats, tags, mask, transitions, start_transitions, stop_transitions):
    from concourse.bass_utils import run_bass_kernel_spmd

    feats = np.asarray(feats, dtype=np.float32)
    tags = np.asarray(tags).astype(np.int64)
    Tm = np.asarray(transitions, dtype=np.float32)
    st = np.asarray(start_transitions, dtype=np.float32)
    sp = np.asarray(stop_transitions, dtype=np.float32)

    gold = _host_gold(feats, tags, Tm, st, sp)

    M = np.exp(Tm.astype(np.float64))             # M[next, cur]
    segs = _segments()
    Ws = [w for _, w, _ in UNIT_SPEC]
    Ls = [L for _, _, L in UNIT_SPEC]
    qoff = np.cumsum([P2] + [Ws[u] * (Ls[u] - 1)
                             for u in range(NUNIT)]).tolist()
    loff = np.cumsum([0] + [2 * w for w in Ws]).tolist()

    # ---- host: exact prefix t=1..JPRE (f64, all batch) + lz prefix ----
    f64 = feats.astype(np.float64)
    P = np.exp(st.astype(np.float64))[:, None] * np.exp(f64[:, 0]).T  # [T,B]
    lz = np.zeros(B)
    s0 = P.sum(0)
    lz += np.log(s0)
    P /= s0
    for t in range(1, JPRE + 1):
        P = np.exp(f64[:, t]).T * (M @ P)
        s0 = P.sum(0)
        lz += np.log(s0)
        P /= s0

    # ---- host: per-segment inits (Perron + warmup + fold, f64) ----
    # Warmup estimates the direction only (those transitions belong to the
    # previous segment's device span); the fold transition fold_t is applied
    # here and its growth counted into lz.
    v = np.ones(T)
    for _ in range(100):
        v = M @ v
        v /= v.sum()
    FP8 = ml_dtypes.float8_e4m3fn
    init_pack = np.empty((NCORES, P2, OUTCOLS), dtype=FP8)
    for (u, j, h, ft, L) in segs:
        if ft == JPRE + 1:
            Pk = P.copy()                          # exact prefix state
        else:
            Pk = np.repeat(v[:, None], B, 1)
            for t in range(ft - WHOST, ft):
                Pk = np.exp(f64[:, t]).T * (M @ Pk)
                Pk /= Pk.sum(0)
        Pk = np.exp(f64[:, ft]).T * (M @ Pk)       # fold transition
        s0 = Pk.sum(0)
        lz += np.log(s0)
        # max-normalize and ship fp8; subtract the exact shipped colsum so
        # quantization + rescale cancel in the telescoping
        shipped = (Pk / Pk.max(0)).astype(FP8)
        lz -= np.log(shipped.astype(np.float64).sum(0))
        blk = shipped.T                            # [B, T]
        for c in range(NCORES):
            init_pack[c, 48 * h:48 * h + 48, UOFF[u] + j * BC:
                      UOFF[u] + (j + 1) * BC] = blk[c * BC:(c + 1) * BC].T

    # first device transition per (unit, block, half)
    starts = [np.zeros((Ws[u] // BC, 2), dtype=np.int64) for u in range(NUNIT)]
    for (u, j, h, ft, L) in segs:
        starts[u][j, h] = ft + 1

    e2 = np.zeros((P2, P2), dtype=np.float32)
    expT = np.exp(Tm).T                            # lhsT[i,j] = exp(T)[j,i]
    e2[:T, :T] = expT
    e2[T:, T:] = expT
    e2 = e2.astype(BF16)

    in_maps = []
    for c in range(NCORES):
        fc = feats[c * BC:(c + 1) * BC]            # [BC, S, T]
        qf = np.exp(fc - DELTA).astype(BF16)       # [BC, S, T]
        qf0 = np.exp(fc).astype(FP8)               # slot-0 q, DELTA0 = 0
        q_pack = np.empty((P2, P2 + QCOLS), dtype=BF16)
        q_pack[:, 0:P2] = e2
        lead = np.zeros((P2, LEADW), dtype=FP8)
        for u in range(NUNIT):
            w = Ws[u]
            for s in range(Ls[u] - 1):
                ts = starts[u] + s                 # [nblk, 2]
                blk = qf[:, ts, :]                 # [BC, nblk, 2, T]
                q_pack[:, qoff[u] + s * w:qoff[u] + (s + 1) * w] = (
                    blk.transpose(2, 3, 1, 0).reshape(P2, w))
            lead[:, loff[u]:loff[u] + w] = init_pack[c, :,
                                                     UOFF[u]:UOFF[u] + w]
            blk0 = qf0[:, starts[u], :]            # [BC, nblk, 2, T]
            lead[:, loff[u] + w:loff[u] + 2 * w] = (
                blk0.transpose(2, 3, 1, 0).reshape(P2, w))
        in_maps.append(dict(q=np.ascontiguousarray(q_pack), lead=lead))

    nc = _get_nc()
    res = run_bass_kernel_spmd(nc, in_maps, list(range(NCORES))).results

    # ---- host: stitch logZ ----
    expsp = np.exp(sp.astype(np.float64))
    last_ft = max(ft for (_, _, _, ft, L) in segs)
    for c in range(NCORES):
        outs = np.asarray(res[c]["outs"]).astype(np.float64)  # [96, OUTCOLS]
        acc = np.zeros(BC)
        for (u, j, h, ft, L) in segs:
            raw = outs[48 * h:48 * h + 48,
                       UOFF[u] + j * BC:UOFF[u] + (j + 1) * BC]  # [T, BC]
            tl = ft + L                        # last device transition
            qv = np.exp(f64[c * BC:(c + 1) * BC, tl, :] - DELTA).T
            F = qv * raw
            if ft == last_ft:
                acc += np.log(expsp @ F)
            else:
                acc += np.log(F.sum(0))
        lz[c * BC:(c + 1) * BC] += acc
    lz += DELTA * (DEV_TRANS - NSEG)   # slot-0 q used DELTA0 = 0

    loss = np.mean(lz - gold)
    return np.float32(loss)


# revision 7
# speedup vs baseline: 1.0048x; 1.0048x over previous
"""CRF negative log-likelihood on 8 Trainium2 NeuronCores — v2.

Spliced forward algorithm, exp domain:  P_{t+1} = q_{t+1} * (M P_t)  with
q = exp(feat - DELTA), M = exp(transitions).  The positive-matrix product
contracts fast (Hilbert metric), so the 511 transitions split into ~64
segments whose initial directions the HOST computes (Perron vector + 2
warmup steps + 1 folded transition, f64); per-batch logZ is stitched on
the host from each segment's final state (rank-1 splicing, validated
err ~3e-3 against exact; tolerance ~49 absolute).

Device work per chain slot per unit (states bf16, 96 partitions = 2
segments stacked; free cols = blocks x 128 batch):
  PE   matmul [96x96 blockdiag exp(T)] @ [96,<=512] -> PSUM f32
  then one of two evacuation paths (DVE/Act capacity split):
   D:  DVE tensor_tensor (PSUM x q) -> SBUF bf16            (direct)
   A:  Act copy PSUM -> SBUF bf16; DVE bf16 fast mult by q  (copy+defer)

1 D-unit [96,1024] (L=9) + 3 A-units [96,1024] (L=6), PSUM exactly
8 banks.  q is precomputed on host (exp + layout) and streamed bf16 in
2-slot chunks; per-unit fp8 [init | q-slot0] lead DMAs (max-normalized
inits, DELTA0=0 q-slot0, exact scale bookkeeping host-side) halve the
serialized DMA prologue so every chain starts early; each unit's final
slot ships the raw (pre q-mult) state and the host applies the last q
during stitching, saving the final evacuation mults.
Gold path score and the final mean are host-side, as in the baseline.
"""

import numpy as np
import ml_dtypes

B, S, T = 1024, 512, 48
NCORES = 8
BC = B // NCORES          # 128 batch rows per core
DELTA = 5.0
P2 = 2 * T                # 96 partitions = 2 segments stacked

# units: (kind, width, L_device_slots); 'D' = DVE direct mult,
# 'A' = Act copy + deferred DVE mult.  A unit of width w holds w/64
# segments (2 stacked x w/128 blocks); each segment covers 1 host-folded
# transition + L device transitions.
UNIT_SPEC = [("D", 1024, 9),
             ("A", 1024, 6), ("A", 1024, 6), ("A", 1024, 6)]
NUNIT = len(UNIT_SPEC)
DEV_TRANS = sum((w // 64) * L for _, w, L in UNIT_SPEC)
COVER = sum((w // 64) * (L + 1) for _, w, L in UNIT_SPEC)
JPRE = (S - 1) - COVER                       # host prefix transitions
QCOLS = sum(w * (L - 1) for _, w, L in UNIT_SPEC)
UOFF = [0]
for _k, _w, _L in UNIT_SPEC:
    UOFF.append(UOFF[-1] + _w)
OUTCOLS = UOFF[-1]
LEADW = P2 + sum(2 * w for _, w, _L in UNIT_SPEC)  # fp8 [e2|init|q0 x u]
NSEG = sum(w // 64 for _, w, _L in UNIT_SPEC)
WHOST = 2                 # host warmup steps per segment init
assert JPRE >= 0, (JPRE, COVER)

BF16 = ml_dtypes.bfloat16

_NC = None
_SEG = None


def _segments():
    """seg_id -> (unit, block j, half h, fold_t, L). seg order: unit-major,
    then block, then half.  Transition fold_t is applied on the host (its
    growth counted there); device applies fold_t+1 .. fold_t+L."""
    global _SEG
    if _SEG is None:
        segs = []
        t0 = JPRE + 1
        for u, (_, w, L) in enumerate(UNIT_SPEC):
            for j in range(w // BC):
                for h in range(2):
                    segs.append((u, j, h, t0, L))
                    t0 += L + 1
        assert t0 == S, (t0, S)
        _SEG = segs
    return _SEG


def _build_nc():
    import concourse.mybir as mybir
    import concourse.tile as tile
    from concourse import bacc

    f32 = mybir.dt.float32
    bf16 = mybir.dt.bfloat16
    Act = mybir.ActivationFunctionType
    Alu = mybir.AluOpType

    nc = bacc.Bacc()

    fp8 = mybir.dt.float8e4
    # leads carry e2 (fp8 stationary) + per-unit [init|q0], all fp8
    q_d = nc.declare_dram_parameter("q", [P2, QCOLS], bf16, isOutput=False)
    lead_d = nc.declare_dram_parameter("lead", [P2, LEADW], fp8,
                                       isOutput=False)
    # final outputs are raw (pre q-mult) states: Act copies PSUM -> SBUF
    # bf16, DMA ships them, host applies the last q during stitching --
    # saves every unit's final DVE mult
    outs_d = nc.declare_dram_parameter("outs", [P2, OUTCOLS], bf16,
                                       isOutput=True)

    Ks = [k for k, _, _ in UNIT_SPEC]
    Ws = [w for _, w, _ in UNIT_SPEC]
    Ls = [L for _, _, L in UNIT_SPEC]
    # device consumes q only for slots 0..L-2 (last slot ships raw PSUM)
    qoff = np.cumsum([0] + [Ws[u] * (Ls[u] - 1)
                            for u in range(NUNIT)]).tolist()
    loff = np.cumsum([P2] + [2 * w for w in Ws]).tolist()
    CHUNK = 4  # slots per q DMA

    with tile.TileContext(nc) as tc:
        with (
            tc.tile_pool(name="const", bufs=1) as cpool,
            tc.tile_pool(name="qp", bufs=1) as qpool,
            tc.tile_pool(name="st", bufs=2) as spool,
            tc.tile_pool(name="raw", bufs=2) as rpool,
            tc.tile_pool(name="psum", bufs=1, space="PSUM") as psum,
        ):
            # fp8 lead tile [e2 | (init_u, q0_u) x units]; 1 DMA per unit
            lead_sb = cpool.tile([P2, LEADW], fp8, name="lead_sb")
            e2_sb = lead_sb[:, 0:P2]

            # static q tiles, one per unit, chunk-DMA'd into subranges
            q_sb = [qpool.tile([P2, Ls[u] * Ws[u]], bf16, tag=f"q{u}",
                               name=f"q{u}") for u in range(NUNIT)]

            def load_q(u, lo, hi):
                hi = min(hi, Ls[u] - 1)
                if lo >= hi:
                    return
                w = Ws[u]
                nc.sync.dma_start(
                    q_sb[u][:, lo * w:hi * w],
                    q_d[:, qoff[u] + lo * w:qoff[u] + hi * w])

            # unit 0's lead carries e2 too
            nc.sync.dma_start(lead_sb[:, 0:loff[1]], lead_d[:, 0:loff[1]])
            for u in range(1, NUNIT):
                nc.sync.dma_start(lead_sb[:, loff[u]:loff[u + 1]],
                                  lead_d[:, loff[u]:loff[u + 1]])
            for u in range(NUNIT):
                load_q(u, 1, 2)
            for u in range(NUNIT):
                load_q(u, 2, 3)
            nchunk = max((L + CHUNK - 1) // CHUNK for L in Ls)
            for c in range(nchunk):
                for u in range(NUNIT):
                    load_q(u, 3 + c * CHUNK, 3 + (c + 1) * CHUNK)

            pt = [psum.tile([P2, Ws[u]], f32, name=f"p{u}")
                  for u in range(NUNIT)]

            states = [lead_sb[:, loff[u]:loff[u] + Ws[u]]
                      for u in range(NUNIT)]

            maxL = max(Ls)
            for s in range(maxL):
                for u in range(NUNIT):
                    kind, w, L = UNIT_SPEC[u]
                    if s >= L:
                        continue
                    prev = states[u]
                    for r in range(0, w, 512):
                        r2 = min(r + 512, w)
                        nc.tensor.matmul(pt[u][:, r:r2], e2_sb[:, :],
                                         prev[:, r:r2], start=True, stop=True)
                    if s == L - 1:
                        # last slot: Act-copy raw out; host applies final q
                        rw = rpool.tile([P2, w], bf16, tag=f"rw{u}",
                                        name=f"rw{u}_fin")
                        nc.scalar.activation(rw[:, :], pt[u][:, :], Act.Copy)
                        nc.sync.dma_start(outs_d[:, UOFF[u]:UOFF[u + 1]],
                                          rw[:, :])
                        continue
                    if s == 0:
                        qs = lead_sb[:, loff[u] + w:loff[u] + 2 * w]
                    else:
                        qs = q_sb[u][:, s * w:(s + 1) * w]
                    stt = spool.tile([P2, w], bf16, tag=f"st{u}",
                                     name=f"st{u}_{s}")
                    if kind == "D":
                        nc.vector.tensor_tensor(stt[:, :], pt[u][:, :], qs,
                                                Alu.mult)
                    else:
                        rw = rpool.tile([P2, w], bf16, tag=f"rw{u}",
                                        name=f"rw{u}_{s}")
                        nc.scalar.activation(rw[:, :], pt[u][:, :], Act.Copy)
                        nc.vector.tensor_tensor(stt[:, :], rw[:, :], qs,
                                                Alu.mult)
                    states[u] = stt

    if not nc.is_finalized():
        nc.finalize()
    return nc


def _get_nc():
    global _NC
    if _NC is None:
        _NC = _build_nc()
    return _NC


def _host_gold(feats, tags, Tm, st, sp):
    emit = np.take_along_axis(feats, tags[..., None], axis=2)[..., 0]
    gold = (
        emit.sum(axis=1, dtype=np.float64)
        + Tm[tags[:, 1:], tags[:, :-1]].sum(axis=1, dtype=np.float64)
        + st[tags[:, 0]].astype(np.float64)
        + sp[tags[:, -1]].astype(np.float64)
    )
    return gold


def kernel(feats, tags, mask, transitions, start_transitions, stop_transitions):
    from concourse.bass_utils import run_bass_kernel_spmd

    feats = np.asarray(feats, dtype=np.float32)
    tags = np.asarray(tags).astype(np.int64)
    Tm = np.asarray(transitions, dtype=np.float32)
    st = np.asarray(start_transitions, dtype=np.float32)
    sp = np.asarray(stop_transitions, dtype=np.float32)

    gold = _host_gold(feats, tags, Tm, st, sp)

    M = np.exp(Tm.astype(np.float64))             # M[next, cur]
    segs = _segments()
    Ws = [w for _, w, _ in UNIT_SPEC]
    Ls = [L for _, _, L in UNIT_SPEC]
    qoff = np.cumsum([0] + [Ws[u] * (Ls[u] - 1)
                            for u in range(NUNIT)]).tolist()
    loff = np.cumsum([P2] + [2 * w for w in Ws]).tolist()

    # ---- host: exact prefix t=1..JPRE (f64, all batch) + lz prefix ----
    f64 = feats.astype(np.float64)
    P = np.exp(st.astype(np.float64))[:, None] * np.exp(f64[:, 0]).T  # [T,B]
    lz = np.zeros(B)
    s0 = P.sum(0)
    lz += np.log(s0)
    P /= s0
    for t in range(1, JPRE + 1):
        P = np.exp(f64[:, t]).T * (M @ P)
        s0 = P.sum(0)
        lz += np.log(s0)
        P /= s0

    # ---- host: per-segment inits (Perron + warmup + fold, f64) ----
    # Warmup estimates the direction only (those transitions belong to the
    # previous segment's device span); the fold transition fold_t is applied
    # here and its growth counted into lz.
    v = np.ones(T)
    for _ in range(100):
        v = M @ v
        v /= v.sum()
    FP8 = ml_dtypes.float8_e4m3fn
    init_pack = np.empty((NCORES, P2, OUTCOLS), dtype=FP8)
    for (u, j, h, ft, L) in segs:
        if ft == JPRE + 1:
            Pk = P.copy()                          # exact prefix state
        else:
            Pk = np.repeat(v[:, None], B, 1)
            for t in range(ft - WHOST, ft):
                Pk = np.exp(f64[:, t]).T * (M @ Pk)
                Pk /= Pk.sum(0)
        Pk = np.exp(f64[:, ft]).T * (M @ Pk)       # fold transition
        s0 = Pk.sum(0)
        lz += np.log(s0)
        # max-normalize and ship fp8; subtract the exact shipped colsum so
        # quantization + rescale cancel in the telescoping
        shipped = (Pk / Pk.max(0)).astype(FP8)
        lz -= np.log(shipped.astype(np.float64).sum(0))
        blk = shipped.T                            # [B, T]
        for c in range(NCORES):
            init_pack[c, 48 * h:48 * h + 48, UOFF[u] + j * BC:
                      UOFF[u] + (j + 1) * BC] = blk[c * BC:(c + 1) * BC].T

    # first device transition per (unit, block, half)
    starts = [np.zeros((Ws[u] // BC, 2), dtype=np.int64) for u in range(NUNIT)]
    for (u, j, h, ft, L) in segs:
        starts[u][j, h] = ft + 1

    e2 = np.zeros((P2, P2), dtype=np.float32)
    expT = np.exp(Tm).T                            # lhsT[i,j] = exp(T)[j,i]
    e2[:T, :T] = expT
    e2[T:, T:] = expT
    e2 = e2.astype(BF16)

    in_maps = []
    for c in range(NCORES):
        fc = feats[c * BC:(c + 1) * BC]            # [BC, S, T]
        qf = np.exp(fc - DELTA).astype(BF16)       # [BC, S, T]
        qf0 = np.exp(fc).astype(FP8)               # slot-0 q, DELTA0 = 0
        q_pack = np.empty((P2, QCOLS), dtype=BF16)
        lead = np.zeros((P2, LEADW), dtype=FP8)
        lead[:, 0:P2] = e2.astype(np.float32).astype(FP8)
        for u in range(NUNIT):
            w = Ws[u]
            for s in range(Ls[u] - 1):
                ts = starts[u] + s                 # [nblk, 2]
                blk = qf[:, ts, :]                 # [BC, nblk, 2, T]
                q_pack[:, qoff[u] + s * w:qoff[u] + (s + 1) * w] = (
                    blk.transpose(2, 3, 1, 0).reshape(P2, w))
            lead[:, loff[u]:loff[u] + w] = init_pack[c, :,
                                                     UOFF[u]:UOFF[u] + w]
            blk0 = qf0[:, starts[u], :]            # [BC, nblk, 2, T]
            lead[:, loff[u] + w:loff[u] + 2 * w] = (
                blk0.transpose(2, 3, 1, 0).reshape(P2, w))
        in_maps.append(dict(q=np.ascontiguousarray(q_pack), lead=lead))

    nc = _get_nc()
    res = run_bass_kernel_spmd(nc, in_maps, list(range(NCORES))).results

    # ---- host: stitch logZ ----
    expsp = np.exp(sp.astype(np.float64))
    last_ft = max(ft for (_, _, _, ft, L) in segs)
    for c in range(NCORES):
        outs = np.asarray(res[c]["outs"]).astype(np.float64)  # [96, OUTCOLS]
        acc = np.zeros(BC)
        for (u, j, h, ft, L) in segs:
            raw = outs[48 * h:48 * h + 48,
                       UOFF[u] + j * BC:UOFF[u] + (j + 1) * BC]  # [T, BC]
            tl = ft + L                        # last device transition
            qv = np.exp(f64[c * BC:(c + 1) * BC, tl, :] - DELTA).T
            F = qv * raw
            if ft == last_ft:
                acc += np.log(expsp @ F)
            else:
                acc += np.log(F.sum(0))
        lz[c * BC:(c + 1) * BC] += acc
    lz += DELTA * (DEV_TRANS - NSEG)   # slot-0 q used DELTA0 = 0

    loss = np.mean(lz - gold)
    return np.float32(loss)
